# revision 38
# baseline (speedup 1.0000x reference)
"""Distributed Trainium2 kernel for a full attention block (QKV proj + RoPE +
bidirectional SDPA + output proj), SPMD across 8 NeuronCores.

Sharding: tensor-parallel over heads (16 heads -> 2 per core) for QKV+attention;
the output projection is column-sharded (each core owns 256 of the 2048 output
channels) over the AllGather'ed attention output, so no core ever needs a
rank-dependent address.

Layouts (all chosen so no on-device transposes are needed):
  - host pre-transposes x -> xT [C, B*T] and all weights -> [in, out]
  - q,k are produced directly in transposed form qT/kT [d, t] by using the
    weight as the stationary matmul operand (v in [t, d] form by swapping roles)
  - attention is computed as scoresT [tk, tq] = (kT-tile).T @ qT, softmax along
    the partition axis: exp on ACT (max-subtraction skipped: inputs are
    unit-normal so |score| <~ 6, safe), denominator via a DVE f16 running sum
    + a ones-matmul partition reduction; the division is applied after the
    attn@v matmul via a gpsimd partition-broadcast reciprocal.

Perf notes (from trace analysis; baseline 536us -> ~466us):
  - PE is the bottleneck: ~790k moving matmul columns at the power-capped
    13/16 clock (1.95 GHz) is a ~405-425us floor; everything else (ACT exp,
    DVE, DMA, collectives) must hide under it, and any PE idle gap >3.4us
    additionally drops the clock to 4/8 (HAM re-throttle).
  - RoPE'd q/k stay in SBUF (qk_sb), no DRAM spill round-trip; rope runs in
    f16 on DVE straight out of PSUM.
  - First x/wqk window is DMA'd in graded chunks (128KB first) so the first
    matmul starts ~2us after the ~11us framework preamble.
  - softmax denominator: f16 ssum on DVE (2x rate) + ones-matmul partition
    reduction; reciprocal_approx_fast (~51 ULP, 5x faster than the
    iterative reciprocal) frees the shared "sr" psum bank in ~1us.
  - AllGathers are per (batch, tq-half): AG cost is floor-dominated
    (~15-25us regardless of 1-4MB), so few big gathers win over many small
    ones (8- and 12-way splits measured strictly worse).
  - emission order = scheduler priority: all attention blocks first (the
    ACT exp chain and AG triggers are the critical path), projection strips
    last — the Tile scheduler pops them as PE filler during exp-gated
    stalls and the AllGather waits. Batch-1 runs hf=1's blocks before
    hf=0's so the hf=1 AllGather overlaps hf=0's exp, and each AG's wait
    is filled by the other half's projection strips.
  - output is written f16 (absmax ~0.6, ~5e-4 rms) to halve tail DMA
    traffic competing with the final AllGather; host upcasts to f32.
"""
import sys
for _p in ("/opt/trn_rl_repo",):
    if _p not in sys.path:
        sys.path.append(_p)

import numpy as np

B, T, C = 2, 2048, 2048
H, D = 16, 128
NCORES = 8
HL = H // NCORES          # heads per core = 2
TT = B * T                # 4096
NKC = C // 128            # 16 contraction chunks
TW = 512                  # t-window (psum bank width in f32)
TW2 = 1024                # tq half (attention block query span)
NTWB = T // TW            # 4 x-windows per batch
NTC = T // 128            # 16 tk chunks per batch
SCALE = float(1.0 / np.sqrt(D))

_CACHE = {}


def _build():
    from concourse import bacc, mybir, tile

    f32 = mybir.dt.float32
    f16 = mybir.dt.float16
    EXP = mybir.ActivationFunctionType.Exp

    nc = bacc.Bacc("TRN2", target_bir_lowering=False, debug=False,
                   num_devices=NCORES)

    xT_ext = nc.dram_tensor("xT", [C, TT], f16, kind="ExternalInput")
    wqk_ext = nc.dram_tensor("wqkT", [C, 4 * 128], f16, kind="ExternalInput")
    wv_ext = nc.dram_tensor("wvT", [C, HL * 128], f16, kind="ExternalInput")
    wp_ext = nc.dram_tensor("wpT", [C, 256], f16, kind="ExternalInput")
    cos_ext = nc.dram_tensor("cosT", [128, T], f16, kind="ExternalInput")
    sin_ext = nc.dram_tensor("sinTs", [128, T], f16, kind="ExternalInput")
    out_ext = nc.dram_tensor("outT", [256, TT], f16, kind="ExternalOutput")

    with tile.TileContext(nc) as tc:
        with tc.tile_pool(name="dram", bufs=1, space="DRAM") as dram:
            # y / allgather per (batch, tq-half): AG cost is floor-dominated
            # (~15-20us regardless of 1-4MB size), so use few big gathers
            y_dram = [[dram.tile([HL * 128, TW2], f16, tag=f"yd{b}{hf}",
                                 name=f"yd{b}{hf}") for hf in range(2)]
                      for b in range(B)]
            ag_dram = [[dram.tile([H * 128, TW2], f16, tag=f"agd{b}{hf}",
                                  name=f"agd{b}{hf}", addr_space="Shared")
                        for hf in range(2)] for b in range(B)]

            with (
                # one PSUM pool, 3 tags, 8 banks total:
                #   mmA: 2-bank slots x2 (wide scores)
                #   mmB: 1-bank x2 (v-proj, attn@v)
                #   sr:  1-bank x2 (qkv accum, denominator, proj accum)
                tc.tile_pool(name="psum", bufs=2, space="PSUM") as psum,
                tc.tile_pool(name="pV", bufs=1) as pV,
            ):
                v_sb = pV.tile([128, TT // 128, HL * 128], f16, tag="v")

                # Pool stack (LIFO close order): pB [attention, whole kernel],
                # pA [x/w slabs, through phase A], pR [rope scratch+tables,
                # phase A only]. pR and pA close before pC (projection) opens.
                pB_cm = tc.tile_pool(name="pB", bufs=1)
                pB = pB_cm.__enter__()
                pA_cm = tc.tile_pool(name="pA", bufs=1)
                pA = pA_cm.__enter__()
                pR_cm = tc.tile_pool(name="pR", bufs=1)
                pR = pR_cm.__enter__()

                # persistent rope'd q/k in SBUF: [128(d), mi, T] per batch,
                # mi in {q_h0, q_h1, k_h0, k_h1}
                qk_sb = [pB.tile([128, 4, T], f16, tag=f"qk{b}",
                                 name=f"qk{b}") for b in range(B)]

                # ---- phase A prologue -------------------------------------
                cos_sb = pR.tile([128, T], f16, tag="cos")
                sin_sb = pR.tile([128, T], f16, tag="sin")
                wqk_sb = pA.tile([128, NKC, 4 * 128], f16, tag="wqk")
                wv_sb = pA.tile([128, NKC, HL * 128], f16, tag="wv")

                def phase_a_window(b, twb):
                    """QKV projection + rope for one 512-wide t window."""
                    tw = b * NTWB + twb
                    x_sb = pA.tile([128, NKC, TW], f16, tag="x", bufs=2,
                                   name="x_sb")
                    if tw == 0:
                        # first window: graded chunks (tiny first so the first
                        # matmul issues ~2us after the framework preamble,
                        # bigger after to keep DMA-issue cost low) in
                        # accumulation order wqk[kc] just ahead of x[kc]
                        for lo, hi in ((0, 1), (1, 2), (2, 4), (4, 8),
                                       (8, 16)):
                            nc.sync.dma_start(
                                wqk_sb[:, lo:hi, :],
                                wqk_ext[lo * 128:hi * 128, :]
                                .rearrange("(kc p) o -> p kc o", p=128))
                            nc.sync.dma_start(
                                x_sb[:, lo:hi, :],
                                xT_ext[lo * 128:hi * 128, 0:TW]
                                .rearrange("(kc p) t -> p kc t", p=128))
                        nc.sync.dma_start(
                            wv_sb[:],
                            wv_ext[:].rearrange("(kc p) o -> p kc o", p=128))
                        nc.sync.dma_start(cos_sb[:], cos_ext[:])
                        nc.sync.dma_start(sin_sb[:], sin_ext[:])
                    else:
                        for q4 in range(4):
                            nc.sync.dma_start(
                                x_sb[:, q4 * 4:(q4 + 1) * 4, :],
                                xT_ext[q4 * 4 * 128:(q4 + 1) * 4 * 128,
                                       tw * TW:(tw + 1) * TW]
                                .rearrange("(kc p) t -> p kc t", p=128))
                    csb = slice(twb * TW, (twb + 1) * TW)
                    for mi in range(4):
                        pqk = psum.tile([128, TW], f32, tag="sr",
                                        name="pqk")
                        for kc in range(NKC):
                            nc.tensor.matmul(
                                pqk[:],
                                wqk_sb[:, kc, mi * 128:(mi + 1) * 128],
                                x_sb[:, kc, :],
                                start=(kc == 0), stop=(kc == NKC - 1))
                        # RoPE: q' = q*cos + swap_halves(q)*sin_signed
                        qraw = pR.tile([128, TW], f16, tag="qraw", bufs=2,
                                       name="qraw")
                        nc.vector.tensor_copy(qraw[:], pqk[:])
                        qrot = pR.tile([128, TW], f16, tag="qrot", bufs=2,
                                       name="qrot")
                        nc.sync.dma_start(qrot[0:64, :], qraw[64:128, :])
                        nc.sync.dma_start(qrot[64:128, :], qraw[0:64, :])
                        qfin = pR.tile([128, TW], f16, tag="qfin", bufs=2,
                                       name="qfin")
                        nc.vector.tensor_mul(qfin[:], qraw[:], cos_sb[:, csb])
                        nc.vector.tensor_mul(qrot[:], qrot[:], sin_sb[:, csb])
                        nc.vector.tensor_add(qk_sb[b][:, mi, csb],
                                             qfin[:], qrot[:])
                    for tci in range(TW // 128):
                        tc_g = tw * (TW // 128) + tci
                        pv = psum.tile([128, HL * 128], f32, tag="sr",
                                       name="pv")
                        for kc in range(NKC):
                            nc.tensor.matmul(
                                pv[:],
                                x_sb[:, kc, tci * 128:(tci + 1) * 128],
                                wv_sb[:, kc, :],
                                start=(kc == 0), stop=(kc == NKC - 1))
                        nc.vector.tensor_copy(v_sb[:, tc_g, :], pv[:])

                # ---- attention helpers ------------------------------------
                ones16 = pB.tile([128, 1], f16, tag="ones16")
                nc.vector.memset(ones16[:], 1.0)

                def all_gather(b, hf):
                    nc.gpsimd.collective_compute(
                        "AllGather",
                        mybir.AluOpType.bypass,
                        replica_groups=[list(range(NCORES))],
                        ins=[y_dram[b][hf][:]],
                        outs=[ag_dram[b][hf][:]],
                    )

                def attn_block(b, hf, h):
                    """scoresT+softmax+attn@v for one (batch, tq-half, head).

                    When h == HL-1, fires the (b, hf) AllGather after the last
                    j-quarter's y lands (the half is then complete).
                    """
                    qh = qk_sb[b][:, h, :]
                    kh = qk_sb[b][:, 2 + h, :]
                    exp_tiles = []
                    ssum = pB.tile([128, TW2], f16, tag="ssum", bufs=2,
                                   name="ssum")
                    for tkc in range(NTC):
                        sc = psum.tile([128, TW2], f32, tag="mmA", name="sc")
                        for j in range(2):
                            tq0 = hf * TW2 + j * TW
                            nc.tensor.matmul(
                                sc[:, j * TW:(j + 1) * TW],
                                kh[:, tkc * 128:(tkc + 1) * 128],
                                qh[:, tq0:tq0 + TW],
                                start=True, stop=True)
                        e = pB.tile([128, TW2], f16, tag=f"e{tkc}",
                                    bufs=2, name=f"e{tkc}")
                        nc.scalar.activation(e[:], sc[:], EXP, scale=SCALE)
                        exp_tiles.append(e)
                        if tkc == 0:
                            nc.vector.tensor_copy(ssum[:], e[:])
                        else:
                            nc.vector.tensor_add(ssum[:], ssum[:], e[:])
                    for j in range(2):
                        py = psum.tile([128, TW], f32, tag="mmB", name="py")
                        for tkc in range(NTC):
                            nc.tensor.matmul(
                                py[:],
                                v_sb[:, b * NTC + tkc, h * 128:(h + 1) * 128],
                                exp_tiles[tkc][:, j * TW:(j + 1) * TW],
                                start=(tkc == 0), stop=(tkc == NTC - 1))
                        ps1 = psum.tile([1, TW], f32, tag="sr", name="ps1")
                        nc.tensor.matmul(ps1[:], ones16[:],
                                         ssum[:, j * TW:(j + 1) * TW],
                                         start=True, stop=True)
                        # fast ~51-ULP reciprocal (denominators are O(10^3),
                        # far from every undefined edge case); ~0.7us vs the
                        # 3.3us iterative reciprocal, frees the psum bank fast
                        recip = pB.tile([1, TW], f32, tag="recip", bufs=2,
                                        name="recip")
                        nc.vector.reciprocal_approx_fast(recip[:], ps1[:])
                        rbs = pB.tile([128, TW], f32, tag="rbs", bufs=2,
                                      name="rbs")
                        nc.gpsimd.partition_broadcast(rbs[:], recip[:])
                        ybf = pB.tile([128, TW], f16, tag="ybf", bufs=2,
                                      name="ybf")
                        nc.vector.tensor_mul(ybf[:], py[:], rbs[:])
                        nc.sync.dma_start(
                            y_dram[b][hf][h * 128:(h + 1) * 128,
                                          j * TW:(j + 1) * TW],
                            ybf[:])
                        if h == HL - 1 and j == 1:
                            all_gather(b, hf)

                # ---- trace schedule ---------------------------------------
                # phase A batch 0 alone (attention has nothing to do yet)
                for twb in range(NTWB):
                    phase_a_window(0, twb)
                # batch-0 attention interleaved with batch-1 phase A windows
                blocks0 = [(hf, h) for hf in range(2) for h in range(HL)]
                for i, twb in enumerate(range(NTWB)):
                    phase_a_window(1, twb)
                    hf, h = blocks0[i]
                    attn_block(0, hf, h)

                # phase A scratch + slabs are dead now
                pR_cm.__exit__(None, None, None)
                pA_cm.__exit__(None, None, None)

                # batch-1 attention with batch-0 projection woven between;
                # strips are emitted after the blocks they can overlap so the
                # scheduler uses them as PE filler while ACT runs exp.
                with tc.tile_pool(name="pC", bufs=1) as pC:
                    wp_sb = pC.tile([128, NKC, 256], f16, tag="wp")
                    nc.sync.dma_start(
                        wp_sb[:],
                        wp_ext[:].rearrange("(kc p) o -> p kc o", p=128))

                    def proj_strip(b, hf, j):
                        ag_sb = pC.tile([128, NKC, TW], f16, tag="ag",
                                        bufs=2, name="ag_sb")
                        # chunked load: the proj accumulation consumes kc in
                        # order, so its first matmuls can start ~1.5us after
                        # the AllGather lands instead of waiting for all 2MB
                        for q4 in range(4):
                            nc.sync.dma_start(
                                ag_sb[:, q4 * 4:(q4 + 1) * 4, :],
                                ag_dram[b][hf][q4 * 4 * 128:(q4 + 1) * 4 * 128,
                                               j * TW:(j + 1) * TW]
                                .rearrange("(kc p) t -> p kc t", p=128))
                        for coc in range(2):
                            po = psum.tile([128, TW], f32, tag="sr",
                                           name="po")
                            for kc in range(NKC):
                                nc.tensor.matmul(
                                    po[:],
                                    wp_sb[:, kc, coc * 128:(coc + 1) * 128],
                                    ag_sb[:, kc, :],
                                    start=(kc == 0), stop=(kc == NKC - 1))
                            od = pC.tile([128, TW], f16, tag="od", bufs=2,
                                         name="od")
                            nc.vector.tensor_copy(od[:], po[:])
                            t0 = b * T + hf * TW2 + j * TW
                            nc.sync.dma_start(
                                out_ext[coc * 128:(coc + 1) * 128,
                                        t0:t0 + TW],
                                od[:])

                    # all blocks first (highest priority: the ACT exp chain
                    # and the AllGather triggers are the critical path), all
                    # strips after — the scheduler pops them as PE filler
                    # during exp-gated stalls and the AllGather waits.
                    # hf=1 first: its AG then overlaps the hf=0 blocks' exp,
                    # and each AG's wait is filled by the other half's strips.
                    attn_block(1, 1, 0)
                    attn_block(1, 1, 1)
                    attn_block(1, 0, 0)
                    attn_block(1, 0, 1)
                    proj_strip(0, 0, 0)
                    proj_strip(0, 0, 1)
                    proj_strip(0, 1, 0)
                    proj_strip(0, 1, 1)
                    # (1,1) strips preload their gathered data (bufs=3) before
                    # the last AG starts and fill the PE during its wait
                    proj_strip(1, 1, 0)
                    proj_strip(1, 1, 1)
                    proj_strip(1, 0, 0)
                    proj_strip(1, 0, 1)

                pB_cm.__exit__(None, None, None)
    nc.compile()
    return nc


def _prepare_in_maps(x, cos, sin, Wqkv, Wproj):
    f16 = np.float16
    xT = np.ascontiguousarray(x.reshape(TT, C).T).astype(f16)
    cosT = np.ascontiguousarray(cos.T).astype(f16)
    sinS = sin.T.astype(np.float32).copy()
    sinS[:D // 2] *= -1.0
    sinTs = np.ascontiguousarray(sinS).astype(f16)
    Wq, Wk, Wv = Wqkv[0:C], Wqkv[C:2 * C], Wqkv[2 * C:3 * C]

    in_maps = []
    for c in range(NCORES):
        hs = [HL * c + j for j in range(HL)]
        wqk_rows = np.concatenate(
            [Wq[h * D:(h + 1) * D] for h in hs]
            + [Wk[h * D:(h + 1) * D] for h in hs], axis=0)
        wv_rows = np.concatenate([Wv[h * D:(h + 1) * D] for h in hs], axis=0)
        in_maps.append({
            "xT": xT,
            "wqkT": np.ascontiguousarray(wqk_rows.T).astype(f16),
            "wvT": np.ascontiguousarray(wv_rows.T).astype(f16),
            "wpT": np.ascontiguousarray(
                Wproj[c * 256:(c + 1) * 256, :].T).astype(f16),
            "cosT": cosT,
            "sinTs": sinTs,
        })
    return in_maps


def run_sharded(x, cos, sin, Wqkv, Wproj, trace=False):
    """Compile (cached), run on 8 cores, return (out, BassKernelResults)."""
    from concourse.bass_utils import run_bass_kernel_spmd

    if "nc" not in _CACHE:
        _CACHE["nc"] = _build()
    nc = _CACHE["nc"]
    in_maps = _prepare_in_maps(x, cos, sin, Wqkv, Wproj)
    res = run_bass_kernel_spmd(nc, in_maps, core_ids=list(range(NCORES)),
                               trace=trace)
    out = np.empty((B, T, C), dtype=np.float32)
    for c in range(NCORES):
        outT = res.results[c]["outT"]          # [256, TT]
        out[:, :, c * 256:(c + 1) * 256] = \
            outT.reshape(256, B, T).transpose(1, 2, 0)
    return out, res


def kernel(x, cos, sin, Wqkv, Wproj):
    out, _ = run_sharded(x, cos, sin, Wqkv, Wproj, trace=False)
    return out


# revision 39
# speedup vs baseline: 1.0285x; 1.0285x over previous
"""Distributed Trainium2 kernel for a full attention block (QKV proj + RoPE +
bidirectional SDPA + output proj), SPMD across 8 NeuronCores.

Sharding: tensor-parallel over heads (16 heads -> 2 per core) for QKV+attention;
the output projection is column-sharded (each core owns 256 of the 2048 output
channels) over the AllGather'ed attention output, so no core ever needs a
rank-dependent address.

Layouts (all chosen so no on-device transposes are needed):
  - host pre-transposes x -> xT [C, B*T] and all weights -> [in, out]
  - q,k are produced directly in transposed form qT/kT [d, t] by using the
    weight as the stationary matmul operand (v in [t, d] form by swapping roles)
  - attention is computed as scoresT [tk, tq] = (kT-tile).T @ qT, softmax along
    the partition axis: exp on ACT (max-subtraction skipped: inputs are
    unit-normal so |score| <~ 6, safe), denominator via a DVE f16 running sum
    + a ones-matmul partition reduction; the division is applied after the
    attn@v matmul via a gpsimd partition-broadcast reciprocal.

Perf notes (from trace analysis; baseline 536us -> ~466us):
  - PE is the bottleneck: ~790k moving matmul columns at the power-capped
    13/16 clock (1.95 GHz) is a ~405-425us floor; everything else (ACT exp,
    DVE, DMA, collectives) must hide under it, and any PE idle gap >3.4us
    additionally drops the clock to 4/8 (HAM re-throttle).
  - RoPE'd q/k stay in SBUF (qk_sb), no DRAM spill round-trip; rope runs in
    f16 on DVE straight out of PSUM.
  - First x/wqk window is DMA'd in graded chunks (128KB first) so the first
    matmul starts ~2us after the ~11us framework preamble.
  - softmax denominator: f16 ssum on DVE (2x rate) + ones-matmul partition
    reduction; reciprocal_approx_fast (~51 ULP, 5x faster than the
    iterative reciprocal) frees the shared "sr" psum bank in ~1us.
  - AllGathers are per (batch, tq-half): AG cost is floor-dominated
    (~15-25us regardless of 1-4MB), so few big gathers win over many small
    ones (8- and 12-way splits measured strictly worse).
  - emission order = scheduler priority: all attention blocks first (the
    ACT exp chain and AG triggers are the critical path), projection strips
    last — the Tile scheduler pops them as PE filler during exp-gated
    stalls and the AllGather waits. Batch-1 runs hf=1's blocks before
    hf=0's so the hf=1 AllGather overlaps hf=0's exp, and each AG's wait
    is filled by the other half's projection strips.
  - output is written f16 (absmax ~0.6, ~5e-4 rms) to halve tail DMA
    traffic competing with the final AllGather; host upcasts to f32.
"""
import sys
for _p in ("/opt/trn_rl_repo",):
    if _p not in sys.path:
        sys.path.append(_p)

import numpy as np

B, T, C = 2, 2048, 2048
H, D = 16, 128
NCORES = 8
HL = H // NCORES          # heads per core = 2
TT = B * T                # 4096
NKC = C // 128            # 16 contraction chunks
TW = 512                  # t-window (psum bank width in f32)
TW2 = 1024                # tq half (attention block query span)
NTWB = T // TW            # 4 x-windows per batch
NTC = T // 128            # 16 tk chunks per batch
SCALE = float(1.0 / np.sqrt(D))

_CACHE = {}


def _build():
    from concourse import bacc, mybir, tile

    f32 = mybir.dt.float32
    f16 = mybir.dt.float16
    EXP = mybir.ActivationFunctionType.Exp

    nc = bacc.Bacc("TRN2", target_bir_lowering=False, debug=False,
                   num_devices=NCORES)

    xT_ext = nc.dram_tensor("xT", [C, TT], f16, kind="ExternalInput")
    wqk_ext = nc.dram_tensor("wqkT", [C, 4 * 128], f16, kind="ExternalInput")
    wv_ext = nc.dram_tensor("wvT", [C, HL * 128], f16, kind="ExternalInput")
    wp_ext = nc.dram_tensor("wpT", [C, 256], f16, kind="ExternalInput")
    cos_ext = nc.dram_tensor("cosT", [128, T], f16, kind="ExternalInput")
    sin_ext = nc.dram_tensor("sinTs", [128, T], f16, kind="ExternalInput")
    out_ext = nc.dram_tensor("outT", [256, TT], f16, kind="ExternalOutput")

    with tile.TileContext(nc) as tc:
        with tc.tile_pool(name="dram", bufs=1, space="DRAM") as dram:
            # y / allgather per (batch, tq-half): AG cost is floor-dominated
            # (~15-20us regardless of 1-4MB size), so use few big gathers
            y_dram = [[dram.tile([HL * 128, TW2], f16, tag=f"yd{b}{hf}",
                                 name=f"yd{b}{hf}") for hf in range(2)]
                      for b in range(B)]
            ag_dram = [[dram.tile([H * 128, TW2], f16, tag=f"agd{b}{hf}",
                                  name=f"agd{b}{hf}", addr_space="Shared")
                        for hf in range(2)] for b in range(B)]

            with (
                # one PSUM pool, 3 tags, 8 banks total:
                #   mmA: 2-bank slots x2 (wide scores)
                #   mmB: 1-bank x2 (v-proj, attn@v)
                #   sr:  1-bank x2 (qkv accum, denominator, proj accum)
                tc.tile_pool(name="psum", bufs=2, space="PSUM") as psum,
                tc.tile_pool(name="pV", bufs=1) as pV,
            ):
                v_sb = pV.tile([128, TT // 128, HL * 128], f16, tag="v")

                # Pool stack (LIFO close order): pB [attention, whole kernel],
                # pA [x/w slabs, through phase A], pR [rope scratch+tables,
                # phase A only]. pR and pA close before pC (projection) opens.
                pB_cm = tc.tile_pool(name="pB", bufs=1)
                pB = pB_cm.__enter__()
                pA_cm = tc.tile_pool(name="pA", bufs=1)
                pA = pA_cm.__enter__()
                pR_cm = tc.tile_pool(name="pR", bufs=1)
                pR = pR_cm.__enter__()

                # persistent rope'd q/k in SBUF: [128(d), mi, T] per batch,
                # mi in {q_h0, q_h1, k_h0, k_h1}
                qk_sb = [pB.tile([128, 4, T], f16, tag=f"qk{b}",
                                 name=f"qk{b}") for b in range(B)]

                # ---- phase A prologue -------------------------------------
                cos_sb = pR.tile([128, T], f16, tag="cos")
                sin_sb = pR.tile([128, T], f16, tag="sin")
                wqk_sb = pA.tile([128, NKC, 4 * 128], f16, tag="wqk")
                wv_sb = pA.tile([128, NKC, HL * 128], f16, tag="wv")

                def phase_a_window(b, twb):
                    """QKV projection + rope for one 512-wide t window."""
                    tw = b * NTWB + twb
                    x_sb = pA.tile([128, NKC, TW], f16, tag="x", bufs=2,
                                   name="x_sb")
                    if tw == 0:
                        # first window: graded chunks (tiny first so the first
                        # matmul issues ~2us after the framework preamble,
                        # bigger after to keep DMA-issue cost low) in
                        # accumulation order wqk[kc] just ahead of x[kc]
                        for lo, hi in ((0, 1), (1, 2), (2, 4), (4, 8),
                                       (8, 16)):
                            nc.sync.dma_start(
                                wqk_sb[:, lo:hi, :],
                                wqk_ext[lo * 128:hi * 128, :]
                                .rearrange("(kc p) o -> p kc o", p=128))
                            nc.sync.dma_start(
                                x_sb[:, lo:hi, :],
                                xT_ext[lo * 128:hi * 128, 0:TW]
                                .rearrange("(kc p) t -> p kc t", p=128))
                        nc.sync.dma_start(
                            wv_sb[:],
                            wv_ext[:].rearrange("(kc p) o -> p kc o", p=128))
                        nc.sync.dma_start(cos_sb[:], cos_ext[:])
                        nc.sync.dma_start(sin_sb[:], sin_ext[:])
                    else:
                        for q4 in range(4):
                            nc.sync.dma_start(
                                x_sb[:, q4 * 4:(q4 + 1) * 4, :],
                                xT_ext[q4 * 4 * 128:(q4 + 1) * 4 * 128,
                                       tw * TW:(tw + 1) * TW]
                                .rearrange("(kc p) t -> p kc t", p=128))
                    csb = slice(twb * TW, (twb + 1) * TW)
                    for mi in range(4):
                        pqk = psum.tile([128, TW], f32, tag="sr",
                                        name="pqk")
                        for kc in range(NKC):
                            nc.tensor.matmul(
                                pqk[:],
                                wqk_sb[:, kc, mi * 128:(mi + 1) * 128],
                                x_sb[:, kc, :],
                                start=(kc == 0), stop=(kc == NKC - 1))
                        # RoPE: q' = q*cos + swap_halves(q)*sin_signed
                        qraw = pR.tile([128, TW], f16, tag="qraw", bufs=2,
                                       name="qraw")
                        nc.vector.tensor_copy(qraw[:], pqk[:])
                        qrot = pR.tile([128, TW], f16, tag="qrot", bufs=2,
                                       name="qrot")
                        nc.sync.dma_start(qrot[0:64, :], qraw[64:128, :])
                        nc.sync.dma_start(qrot[64:128, :], qraw[0:64, :])
                        qfin = pR.tile([128, TW], f16, tag="qfin", bufs=2,
                                       name="qfin")
                        nc.vector.tensor_mul(qfin[:], qraw[:], cos_sb[:, csb])
                        nc.vector.tensor_mul(qrot[:], qrot[:], sin_sb[:, csb])
                        nc.vector.tensor_add(qk_sb[b][:, mi, csb],
                                             qfin[:], qrot[:])
                    for tci in range(TW // 128):
                        tc_g = tw * (TW // 128) + tci
                        pv = psum.tile([128, HL * 128], f32, tag="sr",
                                       name="pv")
                        for kc in range(NKC):
                            nc.tensor.matmul(
                                pv[:],
                                x_sb[:, kc, tci * 128:(tci + 1) * 128],
                                wv_sb[:, kc, :],
                                start=(kc == 0), stop=(kc == NKC - 1))
                        nc.vector.tensor_copy(v_sb[:, tc_g, :], pv[:])

                # ---- attention helpers ------------------------------------
                ones16 = pB.tile([128, 1], f16, tag="ones16")
                nc.vector.memset(ones16[:], 1.0)

                def all_gather(b, hf):
                    nc.gpsimd.collective_compute(
                        "AllGather",
                        mybir.AluOpType.bypass,
                        replica_groups=[list(range(NCORES))],
                        ins=[y_dram[b][hf][:]],
                        outs=[ag_dram[b][hf][:]],
                    )

                def attn_block(b, hf, h):
                    """scoresT+softmax+attn@v for one (batch, tq-half, head).

                    When h == HL-1, fires the (b, hf) AllGather after the last
                    j-quarter's y lands (the half is then complete).
                    """
                    qh = qk_sb[b][:, h, :]
                    kh = qk_sb[b][:, 2 + h, :]
                    exp_tiles = []
                    ssum = pB.tile([128, TW2], f16, tag="ssum", bufs=2,
                                   name="ssum")
                    for tkc in range(NTC):
                        sc = psum.tile([128, TW2], f32, tag="mmA", name="sc")
                        for j in range(2):
                            tq0 = hf * TW2 + j * TW
                            nc.tensor.matmul(
                                sc[:, j * TW:(j + 1) * TW],
                                kh[:, tkc * 128:(tkc + 1) * 128],
                                qh[:, tq0:tq0 + TW],
                                start=True, stop=True)
                        e = pB.tile([128, TW2], f16, tag=f"e{tkc}",
                                    bufs=2, name=f"e{tkc}")
                        nc.scalar.activation(e[:], sc[:], EXP, scale=SCALE)
                        exp_tiles.append(e)
                        if tkc == 0:
                            nc.vector.tensor_copy(ssum[:], e[:])
                        else:
                            nc.vector.tensor_add(ssum[:], ssum[:], e[:])
                    for j in range(2):
                        py = psum.tile([128, TW], f32, tag="mmB", name="py")
                        for tkc in range(NTC):
                            nc.tensor.matmul(
                                py[:],
                                v_sb[:, b * NTC + tkc, h * 128:(h + 1) * 128],
                                exp_tiles[tkc][:, j * TW:(j + 1) * TW],
                                start=(tkc == 0), stop=(tkc == NTC - 1))
                        ps1 = psum.tile([1, TW], f32, tag="sr", name="ps1")
                        nc.tensor.matmul(ps1[:], ones16[:],
                                         ssum[:, j * TW:(j + 1) * TW],
                                         start=True, stop=True)
                        # fast ~51-ULP reciprocal (denominators are O(10^3),
                        # far from every undefined edge case); ~0.7us vs the
                        # 3.3us iterative reciprocal, frees the psum bank fast
                        recip = pB.tile([1, TW], f32, tag="recip", bufs=2,
                                        name="recip")
                        nc.vector.reciprocal_approx_fast(recip[:], ps1[:])
                        rbs = pB.tile([128, TW], f32, tag="rbs", bufs=2,
                                      name="rbs")
                        nc.gpsimd.partition_broadcast(rbs[:], recip[:])
                        ybf = pB.tile([128, TW], f16, tag="ybf", bufs=2,
                                      name="ybf")
                        nc.vector.tensor_mul(ybf[:], py[:], rbs[:])
                        nc.sync.dma_start(
                            y_dram[b][hf][h * 128:(h + 1) * 128,
                                          j * TW:(j + 1) * TW],
                            ybf[:])
                        if h == HL - 1 and j == 1:
                            all_gather(b, hf)

                # ---- trace schedule ---------------------------------------
                # phase A batch 0 alone (attention has nothing to do yet)
                for twb in range(NTWB):
                    phase_a_window(0, twb)
                # batch-0 attention interleaved with batch-1 phase A windows
                blocks0 = [(hf, h) for hf in range(2) for h in range(HL)]
                for i, twb in enumerate(range(NTWB)):
                    phase_a_window(1, twb)
                    hf, h = blocks0[i]
                    attn_block(0, hf, h)

                # phase A scratch + slabs are dead now
                pR_cm.__exit__(None, None, None)
                pA_cm.__exit__(None, None, None)

                # batch-1 attention with batch-0 projection woven between;
                # strips are emitted after the blocks they can overlap so the
                # scheduler uses them as PE filler while ACT runs exp.
                with tc.tile_pool(name="pC", bufs=1) as pC:
                    wp_sb = pC.tile([128, NKC, 256], f16, tag="wp")
                    nc.sync.dma_start(
                        wp_sb[:],
                        wp_ext[:].rearrange("(kc p) o -> p kc o", p=128))

                    def proj_strip(b, hf, j):
                        ag_sb = pC.tile([128, NKC, TW], f16, tag="ag",
                                        bufs=2, name="ag_sb")
                        # chunked load: the proj accumulation consumes kc in
                        # order, so its first matmuls can start ~1.5us after
                        # the AllGather lands instead of waiting for all 2MB
                        for q4 in range(4):
                            nc.sync.dma_start(
                                ag_sb[:, q4 * 4:(q4 + 1) * 4, :],
                                ag_dram[b][hf][q4 * 4 * 128:(q4 + 1) * 4 * 128,
                                               j * TW:(j + 1) * TW]
                                .rearrange("(kc p) t -> p kc t", p=128))
                        for coc in range(2):
                            po = psum.tile([128, TW], f32, tag="sr",
                                           name="po")
                            for kc in range(NKC):
                                nc.tensor.matmul(
                                    po[:],
                                    wp_sb[:, kc, coc * 128:(coc + 1) * 128],
                                    ag_sb[:, kc, :],
                                    start=(kc == 0), stop=(kc == NKC - 1))
                            od = pC.tile([128, TW], f16, tag="od", bufs=2,
                                         name="od")
                            nc.vector.tensor_copy(od[:], po[:])
                            t0 = b * T + hf * TW2 + j * TW
                            nc.sync.dma_start(
                                out_ext[coc * 128:(coc + 1) * 128,
                                        t0:t0 + TW],
                                od[:])

                    # all blocks first (highest priority: the ACT exp chain
                    # and the AllGather triggers are the critical path), all
                    # strips after — the scheduler pops them as PE filler
                    # during exp-gated stalls and the AllGather waits.
                    # hf=1 first: its AG then overlaps the hf=0 blocks' exp,
                    # and each AG's wait is filled by the other half's strips.
                    attn_block(1, 1, 0)
                    attn_block(1, 1, 1)
                    attn_block(1, 0, 0)
                    attn_block(1, 0, 1)
                    proj_strip(0, 0, 0)
                    proj_strip(0, 0, 1)
                    proj_strip(0, 1, 0)
                    proj_strip(0, 1, 1)
                    # (1,1) strips preload their gathered data (bufs=3) before
                    # the last AG starts and fill the PE during its wait
                    proj_strip(1, 1, 0)
                    proj_strip(1, 0, 0)
                    proj_strip(1, 0, 1)
                    # lowest priority: the scheduler holds this strip as the
                    # reserve PE filler for the last AllGather's wait, keeping
                    # the PE warm (no >3.4us gap -> no HAM re-throttle) while
                    # the (1,0) strips wait on their gather
                    proj_strip(1, 1, 1)

                pB_cm.__exit__(None, None, None)
    nc.compile()
    return nc


def _prepare_in_maps(x, cos, sin, Wqkv, Wproj):
    f16 = np.float16
    xT = np.ascontiguousarray(x.reshape(TT, C).T).astype(f16)
    cosT = np.ascontiguousarray(cos.T).astype(f16)
    sinS = sin.T.astype(np.float32).copy()
    sinS[:D // 2] *= -1.0
    sinTs = np.ascontiguousarray(sinS).astype(f16)
    Wq, Wk, Wv = Wqkv[0:C], Wqkv[C:2 * C], Wqkv[2 * C:3 * C]

    in_maps = []
    for c in range(NCORES):
        hs = [HL * c + j for j in range(HL)]
        wqk_rows = np.concatenate(
            [Wq[h * D:(h + 1) * D] for h in hs]
            + [Wk[h * D:(h + 1) * D] for h in hs], axis=0)
        wv_rows = np.concatenate([Wv[h * D:(h + 1) * D] for h in hs], axis=0)
        in_maps.append({
            "xT": xT,
            "wqkT": np.ascontiguousarray(wqk_rows.T).astype(f16),
            "wvT": np.ascontiguousarray(wv_rows.T).astype(f16),
            "wpT": np.ascontiguousarray(
                Wproj[c * 256:(c + 1) * 256, :].T).astype(f16),
            "cosT": cosT,
            "sinTs": sinTs,
        })
    return in_maps


def run_sharded(x, cos, sin, Wqkv, Wproj, trace=False):
    """Compile (cached), run on 8 cores, return (out, BassKernelResults)."""
    from concourse.bass_utils import run_bass_kernel_spmd

    if "nc" not in _CACHE:
        _CACHE["nc"] = _build()
    nc = _CACHE["nc"]
    in_maps = _prepare_in_maps(x, cos, sin, Wqkv, Wproj)
    res = run_bass_kernel_spmd(nc, in_maps, core_ids=list(range(NCORES)),
                               trace=trace)
    out = np.empty((B, T, C), dtype=np.float32)
    for c in range(NCORES):
        outT = res.results[c]["outT"]          # [256, TT]
        out[:, :, c * 256:(c + 1) * 256] = \
            outT.reshape(256, B, T).transpose(1, 2, 0)
    return out, res


def kernel(x, cos, sin, Wqkv, Wproj):
    out, _ = run_sharded(x, cos, sin, Wqkv, Wproj, trace=False)
    return out


# revision 40
# speedup vs baseline: 1.0362x; 1.0075x over previous
"""Distributed Trainium2 kernel for a full attention block (QKV proj + RoPE +
bidirectional SDPA + output proj), SPMD across 8 NeuronCores.

Sharding: tensor-parallel over heads (16 heads -> 2 per core) for QKV+attention;
the output projection is column-sharded (each core owns 256 of the 2048 output
channels) over the AllGather'ed attention output, so no core ever needs a
rank-dependent address.

Layouts (all chosen so no on-device transposes are needed):
  - host pre-transposes x -> xT [C, B*T] and all weights -> [in, out]
  - q,k are produced directly in transposed form qT/kT [d, t] by using the
    weight as the stationary matmul operand (v in [t, d] form by swapping roles)
  - attention is computed as scoresT [tk, tq] = (kT-tile).T @ qT, softmax along
    the partition axis: exp on ACT (max-subtraction skipped: inputs are
    unit-normal so |score| <~ 6, safe), denominator via a DVE f16 running sum
    + a ones-matmul partition reduction; the division is applied after the
    attn@v matmul via a gpsimd partition-broadcast reciprocal.

Perf notes (from trace analysis; baseline 536us -> ~466us):
  - PE is the bottleneck: ~790k moving matmul columns at the power-capped
    13/16 clock (1.95 GHz) is a ~405-425us floor; everything else (ACT exp,
    DVE, DMA, collectives) must hide under it, and any PE idle gap >3.4us
    additionally drops the clock to 4/8 (HAM re-throttle).
  - RoPE'd q/k stay in SBUF (qk_sb), no DRAM spill round-trip; rope runs in
    f16 on DVE straight out of PSUM.
  - First x/wqk window is DMA'd in graded chunks (128KB first) so the first
    matmul starts ~2us after the ~11us framework preamble.
  - softmax denominator: f16 ssum on DVE (2x rate) + ones-matmul partition
    reduction; reciprocal_approx_fast (~51 ULP, 5x faster than the
    iterative reciprocal) frees the shared "sr" psum bank in ~1us.
  - AllGathers are per (batch, tq-half): AG cost is floor-dominated
    (~15-25us regardless of 1-4MB), so few big gathers win over many small
    ones (8- and 12-way splits measured strictly worse).
  - emission order = scheduler priority: all attention blocks first (the
    ACT exp chain and AG triggers are the critical path), projection strips
    last — the Tile scheduler pops them as PE filler during exp-gated
    stalls and the AllGather waits. Batch-1 runs hf=1's blocks before
    hf=0's so the hf=1 AllGather overlaps hf=0's exp, and each AG's wait
    is filled by the other half's projection strips.
  - output is written f16 (absmax ~0.6, ~5e-4 rms) to halve tail DMA
    traffic competing with the final AllGather; host upcasts to f32.
"""
import sys
for _p in ("/opt/trn_rl_repo",):
    if _p not in sys.path:
        sys.path.append(_p)

import numpy as np

B, T, C = 2, 2048, 2048
H, D = 16, 128
NCORES = 8
HL = H // NCORES          # heads per core = 2
TT = B * T                # 4096
NKC = C // 128            # 16 contraction chunks
TW = 512                  # t-window (psum bank width in f32)
TW2 = 1024                # tq half (attention block query span)
NTWB = T // TW            # 4 x-windows per batch
NTC = T // 128            # 16 tk chunks per batch
SCALE = float(1.0 / np.sqrt(D))

_CACHE = {}


def _build():
    from concourse import bacc, mybir, tile

    f32 = mybir.dt.float32
    f16 = mybir.dt.float16
    EXP = mybir.ActivationFunctionType.Exp

    nc = bacc.Bacc("TRN2", target_bir_lowering=False, debug=False,
                   num_devices=NCORES)

    xT_ext = nc.dram_tensor("xT", [C, TT], f16, kind="ExternalInput")
    wqk_ext = nc.dram_tensor("wqkT", [C, 4 * 128], f16, kind="ExternalInput")
    wv_ext = nc.dram_tensor("wvT", [C, HL * 128], f16, kind="ExternalInput")
    wp_ext = nc.dram_tensor("wpT", [C, 256], f16, kind="ExternalInput")
    cos_ext = nc.dram_tensor("cosT", [128, T], f16, kind="ExternalInput")
    sin_ext = nc.dram_tensor("sinTs", [128, T], f16, kind="ExternalInput")
    out_ext = nc.dram_tensor("outT", [256, TT], f16, kind="ExternalOutput")

    with tile.TileContext(nc) as tc:
        with tc.tile_pool(name="dram", bufs=1, space="DRAM") as dram:
            # y / allgather per (batch, tq-half): AG cost is floor-dominated
            # (~15-20us regardless of 1-4MB size), so use few big gathers
            y_dram = [[dram.tile([HL * 128, TW2], f16, tag=f"yd{b}{hf}",
                                 name=f"yd{b}{hf}") for hf in range(2)]
                      for b in range(B)]
            ag_dram = [[dram.tile([H * 128, TW2], f16, tag=f"agd{b}{hf}",
                                  name=f"agd{b}{hf}", addr_space="Shared")
                        for hf in range(2)] for b in range(B)]

            with (
                # one PSUM pool, 3 tags, 8 banks total:
                #   mmA: 2-bank slots x2 (wide scores)
                #   mmB: 1-bank x2 (v-proj, attn@v)
                #   sr:  1-bank x2 (qkv accum, denominator, proj accum)
                tc.tile_pool(name="psum", bufs=2, space="PSUM") as psum,
                tc.tile_pool(name="pV", bufs=1) as pV,
            ):
                v_sb = pV.tile([128, TT // 128, HL * 128], f16, tag="v")

                # Pool stack (LIFO close order): pB [attention, whole kernel],
                # pA [x/w slabs, through phase A], pR [rope scratch+tables,
                # phase A only]. pR and pA close before pC (projection) opens.
                pB_cm = tc.tile_pool(name="pB", bufs=1)
                pB = pB_cm.__enter__()
                pA_cm = tc.tile_pool(name="pA", bufs=1)
                pA = pA_cm.__enter__()
                pR_cm = tc.tile_pool(name="pR", bufs=1)
                pR = pR_cm.__enter__()

                # persistent rope'd q/k in SBUF: [128(d), mi, T] per batch,
                # mi in {q_h0, q_h1, k_h0, k_h1}
                qk_sb = [pB.tile([128, 4, T], f16, tag=f"qk{b}",
                                 name=f"qk{b}") for b in range(B)]

                # ---- phase A prologue -------------------------------------
                cos_sb = pR.tile([128, T], f16, tag="cos")
                sin_sb = pR.tile([128, T], f16, tag="sin")
                wqk_sb = pA.tile([128, NKC, 4 * 128], f16, tag="wqk")
                wv_sb = pA.tile([128, NKC, HL * 128], f16, tag="wv")

                def phase_a_window(b, twb):
                    """QKV projection + rope for one 512-wide t window."""
                    tw = b * NTWB + twb
                    x_sb = pA.tile([128, NKC, TW], f16, tag="x", bufs=2,
                                   name="x_sb")
                    if tw == 0:
                        # first window: graded chunks (tiny first so the first
                        # matmul issues ~2us after the framework preamble,
                        # bigger after to keep DMA-issue cost low) in
                        # accumulation order wqk[kc] just ahead of x[kc]
                        for lo, hi in ((0, 1), (1, 2), (2, 4), (4, 8),
                                       (8, 16)):
                            nc.sync.dma_start(
                                wqk_sb[:, lo:hi, :],
                                wqk_ext[lo * 128:hi * 128, :]
                                .rearrange("(kc p) o -> p kc o", p=128))
                            nc.sync.dma_start(
                                x_sb[:, lo:hi, :],
                                xT_ext[lo * 128:hi * 128, 0:TW]
                                .rearrange("(kc p) t -> p kc t", p=128))
                        nc.sync.dma_start(
                            wv_sb[:],
                            wv_ext[:].rearrange("(kc p) o -> p kc o", p=128))
                        nc.sync.dma_start(cos_sb[:], cos_ext[:])
                        nc.sync.dma_start(sin_sb[:], sin_ext[:])
                    else:
                        for q4 in range(4):
                            nc.sync.dma_start(
                                x_sb[:, q4 * 4:(q4 + 1) * 4, :],
                                xT_ext[q4 * 4 * 128:(q4 + 1) * 4 * 128,
                                       tw * TW:(tw + 1) * TW]
                                .rearrange("(kc p) t -> p kc t", p=128))
                    csb = slice(twb * TW, (twb + 1) * TW)
                    for mi in range(4):
                        pqk = psum.tile([128, TW], f32, tag="sr",
                                        name="pqk")
                        for kc in range(NKC):
                            nc.tensor.matmul(
                                pqk[:],
                                wqk_sb[:, kc, mi * 128:(mi + 1) * 128],
                                x_sb[:, kc, :],
                                start=(kc == 0), stop=(kc == NKC - 1))
                        # RoPE: q' = q*cos + swap_halves(q)*sin_signed
                        qraw = pR.tile([128, TW], f16, tag="qraw", bufs=4,
                                       name="qraw")
                        nc.vector.tensor_copy(qraw[:], pqk[:])
                        qrot = pR.tile([128, TW], f16, tag="qrot", bufs=4,
                                       name="qrot")
                        nc.sync.dma_start(qrot[0:64, :], qraw[64:128, :])
                        nc.sync.dma_start(qrot[64:128, :], qraw[0:64, :])
                        qfin = pR.tile([128, TW], f16, tag="qfin", bufs=2,
                                       name="qfin")
                        nc.vector.tensor_mul(qfin[:], qraw[:], cos_sb[:, csb])
                        nc.vector.tensor_mul(qrot[:], qrot[:], sin_sb[:, csb])
                        nc.vector.tensor_add(qk_sb[b][:, mi, csb],
                                             qfin[:], qrot[:])
                    for tci in range(TW // 128):
                        tc_g = tw * (TW // 128) + tci
                        pv = psum.tile([128, HL * 128], f32, tag="sr",
                                       name="pv")
                        for kc in range(NKC):
                            nc.tensor.matmul(
                                pv[:],
                                x_sb[:, kc, tci * 128:(tci + 1) * 128],
                                wv_sb[:, kc, :],
                                start=(kc == 0), stop=(kc == NKC - 1))
                        nc.vector.tensor_copy(v_sb[:, tc_g, :], pv[:])

                # ---- attention helpers ------------------------------------
                ones16 = pB.tile([128, 1], f16, tag="ones16")
                nc.vector.memset(ones16[:], 1.0)

                def all_gather(b, hf):
                    nc.gpsimd.collective_compute(
                        "AllGather",
                        mybir.AluOpType.bypass,
                        replica_groups=[list(range(NCORES))],
                        ins=[y_dram[b][hf][:]],
                        outs=[ag_dram[b][hf][:]],
                    )

                def attn_block(b, hf, h):
                    """scoresT+softmax+attn@v for one (batch, tq-half, head).

                    When h == HL-1, fires the (b, hf) AllGather after the last
                    j-quarter's y lands (the half is then complete).
                    """
                    qh = qk_sb[b][:, h, :]
                    kh = qk_sb[b][:, 2 + h, :]
                    exp_tiles = []
                    ssum = pB.tile([128, TW2], f16, tag="ssum", bufs=2,
                                   name="ssum")
                    for tkc in range(NTC):
                        sc = psum.tile([128, TW2], f32, tag="mmA", name="sc")
                        for j in range(2):
                            tq0 = hf * TW2 + j * TW
                            nc.tensor.matmul(
                                sc[:, j * TW:(j + 1) * TW],
                                kh[:, tkc * 128:(tkc + 1) * 128],
                                qh[:, tq0:tq0 + TW],
                                start=True, stop=True)
                        e = pB.tile([128, TW2], f16, tag=f"e{tkc}",
                                    bufs=2, name=f"e{tkc}")
                        nc.scalar.activation(e[:], sc[:], EXP, scale=SCALE)
                        exp_tiles.append(e)
                        if tkc == 0:
                            nc.vector.tensor_copy(ssum[:], e[:])
                        else:
                            nc.vector.tensor_add(ssum[:], ssum[:], e[:])
                    for j in range(2):
                        py = psum.tile([128, TW], f32, tag="mmB", name="py")
                        for tkc in range(NTC):
                            nc.tensor.matmul(
                                py[:],
                                v_sb[:, b * NTC + tkc, h * 128:(h + 1) * 128],
                                exp_tiles[tkc][:, j * TW:(j + 1) * TW],
                                start=(tkc == 0), stop=(tkc == NTC - 1))
                        ps1 = psum.tile([1, TW], f32, tag="sr", name="ps1")
                        nc.tensor.matmul(ps1[:], ones16[:],
                                         ssum[:, j * TW:(j + 1) * TW],
                                         start=True, stop=True)
                        # fast ~51-ULP reciprocal (denominators are O(10^3),
                        # far from every undefined edge case); ~0.7us vs the
                        # 3.3us iterative reciprocal, frees the psum bank fast
                        recip = pB.tile([1, TW], f32, tag="recip", bufs=2,
                                        name="recip")
                        nc.vector.reciprocal_approx_fast(recip[:], ps1[:])
                        rbs = pB.tile([128, TW], f32, tag="rbs", bufs=2,
                                      name="rbs")
                        nc.gpsimd.partition_broadcast(rbs[:], recip[:])
                        ybf = pB.tile([128, TW], f16, tag="ybf", bufs=2,
                                      name="ybf")
                        nc.vector.tensor_mul(ybf[:], py[:], rbs[:])
                        nc.sync.dma_start(
                            y_dram[b][hf][h * 128:(h + 1) * 128,
                                          j * TW:(j + 1) * TW],
                            ybf[:])
                        if h == HL - 1 and j == 1:
                            all_gather(b, hf)

                # ---- trace schedule ---------------------------------------
                # phase A batch 0 alone (attention has nothing to do yet)
                for twb in range(NTWB):
                    phase_a_window(0, twb)
                # batch-0 attention interleaved with batch-1 phase A windows
                blocks0 = [(hf, h) for hf in range(2) for h in range(HL)]
                for i, twb in enumerate(range(NTWB)):
                    phase_a_window(1, twb)
                    hf, h = blocks0[i]
                    attn_block(0, hf, h)

                # phase A scratch + slabs are dead now
                pR_cm.__exit__(None, None, None)
                pA_cm.__exit__(None, None, None)

                # batch-1 attention with batch-0 projection woven between;
                # strips are emitted after the blocks they can overlap so the
                # scheduler uses them as PE filler while ACT runs exp.
                with tc.tile_pool(name="pC", bufs=1) as pC:
                    wp_sb = pC.tile([128, NKC, 256], f16, tag="wp")
                    nc.sync.dma_start(
                        wp_sb[:],
                        wp_ext[:].rearrange("(kc p) o -> p kc o", p=128))

                    def proj_strip(b, hf, j):
                        ag_sb = pC.tile([128, NKC, TW], f16, tag="ag",
                                        bufs=2, name="ag_sb")
                        # chunked load: the proj accumulation consumes kc in
                        # order, so its first matmuls can start ~1.5us after
                        # the AllGather lands instead of waiting for all 2MB
                        for q4 in range(4):
                            nc.sync.dma_start(
                                ag_sb[:, q4 * 4:(q4 + 1) * 4, :],
                                ag_dram[b][hf][q4 * 4 * 128:(q4 + 1) * 4 * 128,
                                               j * TW:(j + 1) * TW]
                                .rearrange("(kc p) t -> p kc t", p=128))
                        for coc in range(2):
                            po = psum.tile([128, TW], f32, tag="sr",
                                           name="po")
                            for kc in range(NKC):
                                nc.tensor.matmul(
                                    po[:],
                                    wp_sb[:, kc, coc * 128:(coc + 1) * 128],
                                    ag_sb[:, kc, :],
                                    start=(kc == 0), stop=(kc == NKC - 1))
                            od = pC.tile([128, TW], f16, tag="od", bufs=2,
                                         name="od")
                            nc.vector.tensor_copy(od[:], po[:])
                            t0 = b * T + hf * TW2 + j * TW
                            nc.sync.dma_start(
                                out_ext[coc * 128:(coc + 1) * 128,
                                        t0:t0 + TW],
                                od[:])

                    # all blocks first (highest priority: the ACT exp chain
                    # and the AllGather triggers are the critical path), all
                    # strips after — the scheduler pops them as PE filler
                    # during exp-gated stalls and the AllGather waits.
                    # hf=1 first: its AG then overlaps the hf=0 blocks' exp,
                    # and each AG's wait is filled by the other half's strips.
                    attn_block(1, 1, 0)
                    attn_block(1, 1, 1)
                    attn_block(1, 0, 0)
                    attn_block(1, 0, 1)
                    proj_strip(0, 0, 0)
                    proj_strip(0, 0, 1)
                    proj_strip(0, 1, 0)
                    proj_strip(0, 1, 1)
                    # (1,1) strips preload their gathered data (bufs=3) before
                    # the last AG starts and fill the PE during its wait
                    proj_strip(1, 1, 0)
                    proj_strip(1, 0, 0)
                    proj_strip(1, 0, 1)
                    # lowest priority: the scheduler holds this strip as the
                    # reserve PE filler for the last AllGather's wait, keeping
                    # the PE warm (no >3.4us gap -> no HAM re-throttle) while
                    # the (1,0) strips wait on their gather
                    proj_strip(1, 1, 1)

                pB_cm.__exit__(None, None, None)
    nc.compile()
    return nc


def _prepare_in_maps(x, cos, sin, Wqkv, Wproj):
    f16 = np.float16
    xT = np.ascontiguousarray(x.reshape(TT, C).T).astype(f16)
    cosT = np.ascontiguousarray(cos.T).astype(f16)
    sinS = sin.T.astype(np.float32).copy()
    sinS[:D // 2] *= -1.0
    sinTs = np.ascontiguousarray(sinS).astype(f16)
    Wq, Wk, Wv = Wqkv[0:C], Wqkv[C:2 * C], Wqkv[2 * C:3 * C]

    in_maps = []
    for c in range(NCORES):
        hs = [HL * c + j for j in range(HL)]
        wqk_rows = np.concatenate(
            [Wq[h * D:(h + 1) * D] for h in hs]
            + [Wk[h * D:(h + 1) * D] for h in hs], axis=0)
        wv_rows = np.concatenate([Wv[h * D:(h + 1) * D] for h in hs], axis=0)
        in_maps.append({
            "xT": xT,
            "wqkT": np.ascontiguousarray(wqk_rows.T).astype(f16),
            "wvT": np.ascontiguousarray(wv_rows.T).astype(f16),
            "wpT": np.ascontiguousarray(
                Wproj[c * 256:(c + 1) * 256, :].T).astype(f16),
            "cosT": cosT,
            "sinTs": sinTs,
        })
    return in_maps


def run_sharded(x, cos, sin, Wqkv, Wproj, trace=False):
    """Compile (cached), run on 8 cores, return (out, BassKernelResults)."""
    from concourse.bass_utils import run_bass_kernel_spmd

    if "nc" not in _CACHE:
        _CACHE["nc"] = _build()
    nc = _CACHE["nc"]
    in_maps = _prepare_in_maps(x, cos, sin, Wqkv, Wproj)
    res = run_bass_kernel_spmd(nc, in_maps, core_ids=list(range(NCORES)),
                               trace=trace)
    out = np.empty((B, T, C), dtype=np.float32)
    for c in range(NCORES):
        outT = res.results[c]["outT"]          # [256, TT]
        out[:, :, c * 256:(c + 1) * 256] = \
            outT.reshape(256, B, T).transpose(1, 2, 0)
    return out, res


def kernel(x, cos, sin, Wqkv, Wproj):
    out, _ = run_sharded(x, cos, sin, Wqkv, Wproj, trace=False)
    return out
